# revision 1
# baseline (speedup 1.0000x reference)
"""Trainium2 Bass kernel for nn_DispersiveLoss (B=2048, D=16*768=12288, 8 cores).

Strategy (circulant block decomposition, uniform SPMD):
  x (2048, 12288) -> 16 row-blocks of 128. Core c "owns" m-blocks {2c, 2c+1}
  and computes two Gram strips G[m, m..m+8 (mod 16)] (width 9 blocks = 1152)
  in bf16 with D on partitions (96 k-chunks of 128, PSUM fp32 accumulation).
  Every unordered block pair lands exactly once (circular distance 1..7),
  diagonal blocks are masked to the upper triangle, distance-8 blocks are
  computed twice and weighted 0.5 -- all via one shared mask + ACT scales,
  so the program is identical on all 8 cores (pure SPMD).

  Launch A computes row norms sq_i = ||x_i||^2 for each core's own 256 rows
  (natural layout, DVE/ACT split). The host gathers/redistributes sq (pure
  data movement), then launch B consumes the Gram strips:
    u = d2 - 2D = -2*(g - (sq_n - 2D)/2) + sq_m
  where the per-column term is folded into PSUM by a K=1 ones-matmul and the
  per-partition term rides the ACT bias. ACT produces exp(-u/(D*tau)), u, u^2
  with per-instruction accum_out reductions; DVE handles the triangle-masked
  diagonal blocks. Host combines per-core (128,4) partial sums in float64.
"""

import os

import numpy as np
import ml_dtypes

import concourse.bass as bass
import concourse.mybir as mybir
import concourse.tile as tile
from concourse import bacc
from concourse.bass_utils import run_bass_kernel_spmd

NC_N = 8
B, D = 2048, 12288
BLK = 128
UNION = 1280  # 10 blocks per core in SBUF
STRIPW = 1152  # 9-block strip width
KCH = 96  # k-chunks of 128
KB = 8  # k-chunks per DMA batch
TAU = 0.5
CC = float(2 * D)  # centering constant (E[d2] for N(0,1) rows)
SS = 1.0 / (D * TAU)  # exponent scale
F32 = mybir.dt.float32
BF16 = mybir.dt.bfloat16
USE_FP8 = os.environ.get("KERNEL_NO_FP8", "") == ""
if USE_FP8:
    DT_IN = mybir.dt.float8e4
    NP_IN = ml_dtypes.float8_e4m3
else:
    DT_IN = BF16
    NP_IN = ml_dtypes.bfloat16
LN_HALF = float(np.log(0.5))
INV_SQRT2 = float(1.0 / np.sqrt(2.0))

# effective pair count: 16 * (tri 8128 + 7 full blocks + half block)
N_PAIRS = B * (B - 1) // 2

KERNEL_EXEC_NS = []  # filled when KERNEL_TRACE is set (test harness only)

_cache = {}


def _trace_enabled():
    return bool(os.environ.get("KERNEL_TRACE"))


def _build_sq_kernel():
    """Launch A: per core, sq for its own 256 rows from natural-layout bf16."""
    nc = bacc.Bacc("TRN2", target_bir_lowering=False, debug=False, num_devices=NC_N)
    xn = nc.dram_tensor("xn", [2, BLK, D], DT_IN, kind="ExternalInput")
    sq_out = nc.dram_tensor("sq_out", [2, BLK], F32, kind="ExternalOutput")
    HD = D // 2  # half-block DMA granularity
    NSUB = 4  # compute sub-slices per half

    with tile.TileContext(nc) as tc:
        with (
            tc.tile_pool(name="p", bufs=4) as p,
            tc.tile_pool(name="a", bufs=1) as a,
        ):
            acc = a.tile([BLK, 16], F32)
            # 4 big DMAs (block-half each); DVE handles block 0, ACT block 1
            SUB = HD // NSUB
            for h in range(2):
                for b in range(2):
                    t = p.tile([BLK, HD], DT_IN, tag="xin")
                    nc.sync.dma_start(t[:], xn[b, :, h * HD : (h + 1) * HD])
                    for j in range(NSUB):
                        col_i = b * 8 + h * NSUB + j
                        col = acc[:, col_i : col_i + 1]
                        ts_ = t[:, j * SUB : (j + 1) * SUB]
                        scr = p.tile([BLK, SUB], F32, tag="scr")
                        if b == 0:
                            nc.vector.scalar_tensor_tensor(
                                out=scr[:],
                                in0=ts_,
                                scalar=1.0,
                                in1=ts_,
                                op0=mybir.AluOpType.mult,
                                op1=mybir.AluOpType.mult,
                                accum_out=col,
                            )
                        else:
                            nc.scalar.activation(
                                scr[:],
                                ts_,
                                mybir.ActivationFunctionType.Square,
                                accum_out=col,
                            )
            r = a.tile([BLK, 2], F32)
            nc.vector.tensor_reduce(
                r[:, 0:1], acc[:, 0:8], mybir.AxisListType.X, mybir.AluOpType.add
            )
            nc.vector.tensor_reduce(
                r[:, 1:2], acc[:, 8:16], mybir.AxisListType.X, mybir.AluOpType.add
            )
            for b in range(2):
                nc.sync.dma_start(
                    sq_out[b].rearrange("(p o) -> p o", o=1), r[:, b : b + 1]
                )
    nc.compile()
    return nc


def _build_main_kernel():
    """Launch B: Gram strips + statistics."""
    nc = bacc.Bacc("TRN2", target_bir_lowering=False, debug=False, num_devices=NC_N)
    xT = nc.dram_tensor("xT", [BLK, KCH, UNION], DT_IN, kind="ExternalInput")
    sqw = nc.dram_tensor("sqw", [UNION], F32, kind="ExternalInput")
    tri = nc.dram_tensor("tri", [BLK, BLK], F32, kind="ExternalInput")
    out_stats = nc.dram_tensor("out_stats", [BLK, 4], F32, kind="ExternalOutput")

    MULT = mybir.AluOpType.mult
    ADD = mybir.AluOpType.add
    EXP = mybir.ActivationFunctionType.Exp
    SQUARE = mybir.ActivationFunctionType.Square
    IDENT = mybir.ActivationFunctionType.Identity
    SQRT = mybir.ActivationFunctionType.Sqrt

    # acc columns: 0:6 E [mid1, mid2, half1, half2, tri1, tri2]
    #              6:12 S1, 12:18 S2, 18 feat
    with tile.TileContext(nc) as tc:
        with (
            tc.tile_pool(name="slab", bufs=4) as slab_pool,
            tc.tile_pool(name="psp", bufs=1, space="PSUM") as psp,
            tc.tile_pool(name="post", bufs=2) as post,
            tc.tile_pool(name="accp", bufs=1) as accp,
        ):
            ps0 = psp.tile([BLK, STRIPW], F32, tag="ps0")
            ps1 = psp.tile([BLK, STRIPW], F32, tag="ps1")
            ps = [ps0, ps1]
            acc = accp.tile([BLK, 19], F32)

            # --- early work that only depends on inputs ---
            # PE pre-warm: keep the HAM activity window busy while slab 0 DMAs
            warm = post.tile([BLK, 512], DT_IN, tag="warm")
            nc.gpsimd.memset(warm[:], 0.0)
            wps = psp.tile([BLK, 512], F32, tag="wps")
            for _ in range(14):
                nc.tensor.matmul(
                    wps[:], warm[:, 0:128], warm[:], start=True, stop=True,
                    skip_group_check=True,
                )
            # feat partial: sum sqrt(sq_own) over own 256 rows (input-only dep)
            ft = post.tile([BLK, 2], F32, tag="ft")
            nc.sync.dma_start(ft[:], sqw[0:256].rearrange("(j p) -> p j", j=2))
            fscr = post.tile([BLK, 2], F32, tag="fscr")
            nc.scalar.activation(fscr[:], ft[:], SQRT, accum_out=acc[:, 18:19])
            # preload the Exp activation table AFTER the sqrt so it stays
            # resident for the tail exp ops
            zcol = post.tile([BLK, 1], F32, tag="zcol")
            nc.gpsimd.memset(zcol[:], 0.0)
            zscr = post.tile([BLK, 1], F32, tag="zscr")
            nc.scalar.activation(zscr[:], zcol[:], mybir.ActivationFunctionType.Exp)

            # strip s: m-block at union col 128*s, window = union cols 128*s..128*s+1152
            segs = [(0, 512), (512, 1024), (1024, 1152)]

            KSTEP = 2 if USE_FP8 else 1
            DR = mybir.MatmulPerfMode.DoubleRow if USE_FP8 else None
            for kb in range(KCH // KB):
                st = slab_pool.tile([BLK, KB, UNION], DT_IN, tag="slab")
                nc.sync.dma_start(st[:], xT[:, kb * KB : (kb + 1) * KB, :])
                for ii in range(0, KB, KSTEP):
                    k = kb * KB + ii
                    for s in range(2):
                        off = 128 * s
                        lhs = st[:, ii : ii + KSTEP, off : off + 128]
                        for c0, c1 in segs:
                            nc.tensor.matmul(
                                ps[s][:, c0:c1],
                                lhs,
                                st[:, ii : ii + KSTEP, off + c0 : off + c1],
                                start=(k == 0),
                                stop=False,
                                perf_mode=DR,
                            )

            # ---- post processing ----
            # Fold BOTH rank-1 terms into PSUM with K=1 matmuls so that
            #   p := ps = g - (sq_n - CC)/2 - sq_m/2  =  -(d2 - CC)/2 = -u/2.
            # Stats then read PSUM directly (host rescales: S1 = -2*sum p,
            # S2 = 4*sum p^2; exp(-S*u) = exp(2*S*p) on ACT).
            ones = post.tile([1, 512], F32, tag="ones")
            nc.gpsimd.memset(ones[:], 1.0)
            sqrow = post.tile([1, UNION], F32, tag="sqrow")
            nc.sync.dma_start(sqrow[:], sqw[:].rearrange("(a b) -> a b", a=1))
            vrow = post.tile([1, UNION], F32, tag="vrow")
            # v = (sq - CC) * (-0.5)
            nc.vector.tensor_scalar(
                out=vrow[:],
                in0=sqrow[:],
                scalar1=-CC,
                scalar2=-0.5,
                op0=ADD,
                op1=MULT,
            )
            wrow = post.tile([1, 256], F32, tag="wrow")
            # w = -sq_own/2 for the two m-blocks
            nc.vector.tensor_scalar(
                out=wrow[:], in0=sqrow[:, 0:256], scalar1=-0.5, scalar2=0.0,
                op0=MULT, op1=ADD,
            )
            # per-column: ps += ones^T (1x128) @ v  (shared lhsT)
            for s in range(2):
                off = 128 * s
                for c0, c1 in segs:
                    nc.tensor.matmul(
                        ps[s][:, c0:c1],
                        ones[:, 0:128],
                        vrow[:, off + c0 : off + c1],
                        start=False,
                        stop=False,
                    )
            # per-row: ps += w^T (1x128, = -sq_m/2) @ ones
            for s in range(2):
                for c0, c1 in segs:
                    nc.tensor.matmul(
                        ps[s][:, c0:c1],
                        wrow[:, 128 * s : 128 * s + 128],
                        ones[:, 0 : c1 - c0],
                        start=False,
                        stop=(c0, c1) == segs[-1],
                    )

            tri_t = post.tile([BLK, BLK], F32, tag="tri")
            nc.sync.dma_start(tri_t[:], tri[:])
            lnhalf = post.tile([BLK, 1], F32, tag="lnhalf")
            nc.gpsimd.memset(lnhalf[:], LN_HALF)

            S2E = 2.0 * SS
            for s in range(2):
                p = ps[s]
                # mid region (full-weight blocks): cols 128:1024
                pm = p[:, 128:1024]
                scr = post.tile([BLK, 896], F32, tag="scr")
                nc.scalar.activation(scr[:], pm, EXP, scale=S2E, accum_out=acc[:, s : s + 1])
                # copy PSUM->SBUF with fused S1 accumulation, then square
                # against the PSUM copy (DVE allows only one PSUM operand)
                pmS = post.tile([BLK, 896], F32, tag="pmS")
                nc.vector.tensor_scalar(
                    out=pmS[:], in0=pm, scalar1=1.0, scalar2=0.0,
                    op0=MULT, op1=ADD, accum_out=acc[:, 6 + s : 7 + s],
                )
                scrd = post.tile([BLK, 896], F32, tag="scrd")
                nc.vector.scalar_tensor_tensor(
                    out=scrd[:], in0=pmS[:], scalar=1.0, in1=pm,
                    op0=MULT, op1=MULT, accum_out=acc[:, 12 + s : 13 + s],
                )

                # half-weight region (distance-8 block, computed twice fleet-wide):
                # cols 1024:1152; weight 0.5 folded into scales
                ph = p[:, 1024:1152]
                scr2 = post.tile([BLK, BLK], F32, tag="scr2")
                nc.scalar.activation(
                    scr2[:], ph, EXP, bias=lnhalf[:], scale=S2E,
                    accum_out=acc[:, 2 + s : 3 + s],
                )
                phS = post.tile([BLK, BLK], F32, tag="phS")
                nc.vector.tensor_scalar(
                    out=phS[:], in0=ph, scalar1=0.5, scalar2=0.0,
                    op0=MULT, op1=ADD, accum_out=acc[:, 8 + s : 9 + s],
                )
                scr2d = post.tile([BLK, BLK], F32, tag="scr2d")
                nc.vector.scalar_tensor_tensor(
                    out=scr2d[:], in0=phS[:], scalar=1.0, in1=ph,
                    op0=MULT, op1=MULT, accum_out=acc[:, 14 + s : 15 + s],
                )

                # diagonal block (upper-triangle mask): cols 0:128
                pd = p[:, 0:128]
                et = post.tile([BLK, BLK], F32, tag=f"et{s}")
                nc.scalar.activation(et[:], pd, EXP, scale=S2E)
                me = post.tile([BLK, BLK], F32, tag="me")
                nc.vector.scalar_tensor_tensor(
                    out=me[:], in0=et[:], scalar=1.0, in1=tri_t[:],
                    op0=MULT, op1=MULT, accum_out=acc[:, 4 + s : 5 + s],
                )
                mu = post.tile([BLK, BLK], F32, tag=f"mu{s}")
                nc.vector.scalar_tensor_tensor(
                    out=mu[:], in0=pd, scalar=1.0, in1=tri_t[:],
                    op0=MULT, op1=MULT, accum_out=acc[:, 10 + s : 11 + s],
                )
                ms2 = post.tile([BLK, BLK], F32, tag="ms2")
                nc.vector.scalar_tensor_tensor(
                    out=ms2[:], in0=mu[:], scalar=1.0, in1=pd,
                    op0=MULT, op1=MULT, accum_out=acc[:, 16 + s : 17 + s],
                )

            outt = accp.tile([BLK, 4], F32)
            nc.vector.tensor_reduce(outt[:, 0:1], acc[:, 0:6], mybir.AxisListType.X, ADD)
            nc.vector.tensor_reduce(outt[:, 1:2], acc[:, 6:12], mybir.AxisListType.X, ADD)
            nc.vector.tensor_reduce(outt[:, 2:3], acc[:, 12:18], mybir.AxisListType.X, ADD)
            nc.vector.tensor_copy(outt[:, 3:4], acc[:, 18:19])
            nc.sync.dma_start(out_stats[:], outt[:])
    nc.compile()
    return nc


def _get(name, builder):
    if name not in _cache:
        _cache[name] = builder()
    return _cache[name]


def _run(nc, in_maps, tag):
    if _trace_enabled():
        try:
            import profhook

            profhook.install()
        except Exception:
            pass
        import tempfile

        res = run_bass_kernel_spmd(
            nc, in_maps, list(range(NC_N)), trace=True,
            tmpdir=tempfile.mkdtemp(prefix=f"ktrace_{tag}_"),
        )
        KERNEL_EXEC_NS.append((tag, res.exec_time_ns))
        return res.results
    return run_bass_kernel_spmd(nc, in_maps, list(range(NC_N))).results


def kernel(features):
    x = np.asarray(features).reshape(B, D)
    xbf = x.astype(NP_IN)

    # ---- launch A: row norms ----
    a_maps = [
        {"xn": np.ascontiguousarray(xbf[256 * c : 256 * c + 256]).reshape(2, BLK, D)}
        for c in range(NC_N)
    ]
    nc_a = _get("sq", _build_sq_kernel)
    a_res = _run(nc_a, a_maps, "sq")
    sq_full = np.concatenate([a_res[c]["sq_out"].reshape(256) for c in range(NC_N)])

    # ---- launch B: Gram strips + stats ----
    xT_full = np.ascontiguousarray(xbf.T)  # (D, B)
    b_maps = []
    tri = np.triu(np.ones((BLK, BLK), np.float32), k=1)
    for c in range(NC_N):
        cols = (256 * c + np.arange(UNION)) % B
        xu = xT_full[:, cols].reshape(KCH, BLK, UNION).transpose(1, 0, 2)
        b_maps.append(
            {
                "xT": np.ascontiguousarray(xu),
                "sqw": sq_full[cols].astype(np.float32),
                "tri": tri,
            }
        )
    nc_b = _get("main", _build_main_kernel)
    b_res = _run(nc_b, b_maps, "main")

    # ---- host combine (gather of partial sums only) ----
    E = S1 = S2 = FT = 0.0
    for c in range(NC_N):
        o = b_res[c]["out_stats"].astype(np.float64)
        E += o[:, 0].sum()
        S1 += o[:, 1].sum()
        S2 += o[:, 2].sum()
        FT += o[:, 3].sum()
    # device accumulates p = -u/2 and p^2 (with the 0.5-weighted region folded)
    S1 = -2.0 * S1
    S2 = 4.0 * S2

    N = float(N_PAIRS)
    mean_u = S1 / N
    mean = (mean_u + CC) / D
    var_u = (S2 - N * mean_u * mean_u) / (N - 1.0)
    std = np.sqrt(var_u) / D
    # logsumexp(-pdn/tau) = -CC*SS + log(E); loss = -that + log(N)
    loss = CC * SS - np.log(E) + np.log(N)
    feat_norm = FT / B

    return (
        np.float32(loss),
        np.float32(feat_norm),
        np.float32(mean),
        np.float32(std),
    )


if __name__ == "__main__":
    f = np.random.default_rng(0).standard_normal((B, 16, 768), dtype=np.float32)
    print(kernel(features=f))



# revision 9
# speedup vs baseline: 1.0680x; 1.0680x over previous
"""Trainium2 Bass kernel for nn_DispersiveLoss (B=2048, D=16*768=12288, 8 cores).

Single-launch circulant block decomposition (uniform SPMD):
  x (2048, 12288) -> 16 row-blocks of 128. Core c owns m-blocks {2c, 2c+1}
  and computes two Gram strips G[m, m..m+8 (mod 16)] (width 9 blocks = 1152)
  in fp8 DoubleRow (D on partitions, 48 double-k-chunks, PSUM f32 accum).

  sq_i = ||x_i||^2 for the core's own 256 rows is computed early on the
  otherwise-idle DVE/ACT engines from a natural-layout copy of the rows,
  then u = -(sq - D)/2 (bf16, centered ~N(0,80)) is AllGathered across the
  8 cores on-device and read back rotated via a partition_id-offset dynamic
  DMA. The SAME vector u serves both rank-1 PSUM corrections
  (p = g + u_col + u_row = -(d2 - 2D)/2), folded in with K=1 bf16 matmuls,
  so PSUM holds centered pair values (diag p_ii = D exactly).

  Post-processing per strip is host-weighted-region based (no triangle
  mask): full window W=[0:1152], diag D=[0:128], dist-8 H=[1024:1152] each
  produce (E, S1, S2) partial sums; the host combines with weights
  (1, -1/2, -1/2) in float64 and subtracts the closed-form diagonal
  constants. The strip-1 tail of the k-loop overlaps strip-0's post.
"""

import os

import numpy as np
import ml_dtypes

import concourse.bass as bass
import concourse.mybir as mybir
import concourse.tile as tile
from concourse import bacc
from concourse.bass_utils import run_bass_kernel_spmd

NC_N = 8
B, D = 2048, 12288
BLK = 128
UNION = 1280  # 10 blocks per core in SBUF
STRIPW = 1152  # 9-block strip width
KCH = 96  # k-chunks of 128
KB = 8  # k-chunks per DMA slab
TAU = 0.5
CC = float(2 * D)  # centering constant (E[d2] for N(0,1) rows)
SS = 1.0 / (D * TAU)  # exponent scale
S2E = 2.0 * SS
F32 = mybir.dt.float32
BF16 = mybir.dt.bfloat16
DT_IN = mybir.dt.float8e4
NP_IN = ml_dtypes.float8_e4m3

N_PAIRS = B * (B - 1) // 2

KERNEL_EXEC_NS = []  # filled when KERNEL_TRACE is set (test harness only)

_cache = {}


def _trace_enabled():
    return bool(os.environ.get("KERNEL_TRACE"))


def _build_kernel():
    nc = bacc.Bacc("TRN2", target_bir_lowering=False, debug=False, num_devices=NC_N)
    xT = nc.dram_tensor("xT", [BLK, KCH, UNION], DT_IN, kind="ExternalInput")
    xn = nc.dram_tensor("xn", [2, BLK, D], DT_IN, kind="ExternalInput")
    out_stats = nc.dram_tensor("out_stats", [BLK, 19], F32, kind="ExternalOutput")

    MULT = mybir.AluOpType.mult
    ADD = mybir.AluOpType.add
    EXP = mybir.ActivationFunctionType.Exp
    SQUARE = mybir.ActivationFunctionType.Square
    SQRT = mybir.ActivationFunctionType.Sqrt
    DR = mybir.MatmulPerfMode.DoubleRow
    X = mybir.AxisListType.X

    HD = D // 2  # xn half-block DMA granularity
    segs = [(0, 512), (512, 1024), (1024, 1152)]

    # acc columns: 0 feat; strip s at 1+9s: Ew Ed Eh S1w S1d S1h S2w S2d S2h
    with tile.TileContext(nc) as tc:
        with (
            tc.tile_pool(name="slab", bufs=4) as slab_pool,
            tc.tile_pool(name="xnp", bufs=2) as xnp,
            tc.tile_pool(name="psp", bufs=1, space="PSUM") as psp,
            tc.tile_pool(name="post", bufs=2) as post,
            tc.tile_pool(name="accp", bufs=1) as accp,
            tc.tile_pool(name="dram", bufs=1, space="DRAM") as dram,
        ):
            ps0 = psp.tile([BLK, STRIPW], F32, tag="ps0")
            ps1 = psp.tile([BLK, STRIPW], F32, tag="ps1")
            ps = [ps0, ps1]
            acc = accp.tile([BLK, 19], F32)
            sqa = accp.tile([BLK, 16], F32)
            r = accp.tile([BLK, 2], F32)

            # --- PE pre-warm: trip the HAM busy window while slab 0 DMAs ---
            warm = post.tile([BLK, 512], DT_IN, tag="warm")
            nc.gpsimd.memset(warm[:], 0.0)
            wps = psp.tile([BLK, 512], F32, tag="wps")
            for _ in range(14):
                nc.tensor.matmul(
                    wps[:], warm[:, 0:128], warm[:], start=True, stop=True,
                    skip_group_check=True,
                )
            ones = post.tile([1, 512], BF16, tag="ones")
            nc.gpsimd.memset(ones[:], 1.0)

            KPS = KB // 2  # k-pairs per slab (DoubleRow consumes 2 chunks)
            sts = []
            for kb in range(KCH // KB):
                st = slab_pool.tile([BLK, KB, UNION], DT_IN, tag="slab")
                sts.append(st)
                nc.sync.dma_start(st[:], xT[:, kb * KB : (kb + 1) * KB, :])

                if kb == 0:
                    # --- sq of own 256 rows from natural layout (DVE+ACT) ---
                    NSUB = 4
                    SUB = HD // NSUB
                    for b in range(2):
                        for h in range(2):
                            t = xnp.tile([BLK, HD], DT_IN, tag="xin")
                            nc.sync.dma_start(t[:], xn[b, :, h * HD : (h + 1) * HD])
                            for j in range(NSUB):
                                col_i = (b * 2 + h) * NSUB + j
                                col = sqa[:, col_i : col_i + 1]
                                ts_ = t[:, j * SUB : (j + 1) * SUB]
                                scr = xnp.tile([BLK, SUB], F32, tag="xsc")
                                if b == 0:
                                    nc.vector.scalar_tensor_tensor(
                                        out=scr[:], in0=ts_, scalar=1.0, in1=ts_,
                                        op0=MULT, op1=MULT, accum_out=col,
                                    )
                                else:
                                    nc.scalar.activation(
                                        scr[:], ts_, SQUARE, accum_out=col,
                                    )
                    nc.vector.tensor_reduce(r[:, 0:1], sqa[:, 0:8], X, ADD)
                    nc.vector.tensor_reduce(r[:, 1:2], sqa[:, 8:16], X, ADD)
                    # feat partial: sum sqrt(sq) (ACT Sqrt table, then preload
                    # Exp so the table is resident for the post phase)
                    fscr = accp.tile([BLK, 2], F32)
                    nc.scalar.activation(fscr[:], r[:], SQRT, accum_out=acc[:, 0:1])
                    zcol = accp.tile([BLK, 1], F32)
                    nc.gpsimd.memset(zcol[:], 0.0)
                    zscr = accp.tile([BLK, 1], F32)
                    nc.scalar.activation(zscr[:], zcol[:], EXP)
                    # u = -(sq - D)/2 in bf16; AllGather across cores
                    uown = accp.tile([BLK, 2], BF16)
                    nc.vector.tensor_scalar(
                        out=uown[:], in0=r[:], scalar1=-float(D), scalar2=-0.5,
                        op0=ADD, op1=MULT,
                    )
                    ub = dram.tile([256], BF16)
                    ug = dram.tile([4096], BF16)
                    nc.gpsimd.dma_start(ub[:].rearrange("(b p) -> p b", b=2), uown[:])
                    nc.gpsimd.collective_compute(
                        "AllGather",
                        mybir.AluOpType.bypass,
                        replica_groups=[list(range(NC_N))],
                        ins=[ub[:]],
                        outs=[ug[0:2048]],
                    )
                    nc.gpsimd.dma_start(ug[2048:4096], ug[0:2048])
                    pid = nc.gpsimd.partition_id()
                    urow = accp.tile([1, UNION], BF16)
                    nc.gpsimd.dma_start(
                        urow[:],
                        ug[bass.DynSlice(pid * 256, UNION)].rearrange(
                            "(a b) -> a b", a=1
                        ),
                    )

                if kb < 10:
                    for kp in range(KPS):
                        ii = 2 * kp
                        for s in range(2):
                            off = 128 * s
                            lhs = st[:, ii : ii + 2, off : off + 128]
                            for c0, c1 in segs:
                                nc.tensor.matmul(
                                    ps[s][:, c0:c1],
                                    lhs,
                                    st[:, ii : ii + 2, off + c0 : off + c1],
                                    start=(kb == 0 and kp == 0),
                                    stop=False,
                                    perf_mode=DR,
                                )

            def strip_tail_mms(s):
                off = 128 * s
                for kb in (10, 11):
                    st = sts[kb]
                    for kp in range(KPS):
                        ii = 2 * kp
                        lhs = st[:, ii : ii + 2, off : off + 128]
                        for c0, c1 in segs:
                            nc.tensor.matmul(
                                ps[s][:, c0:c1],
                                lhs,
                                st[:, ii : ii + 2, off + c0 : off + c1],
                                start=False,
                                stop=False,
                                perf_mode=DR,
                            )
                # rank-1 corrections: p += 1^T @ u_col  and  u_row^T @ 1
                for c0, c1 in segs:
                    nc.tensor.matmul(
                        ps[s][:, c0:c1],
                        ones[:, 0:128],
                        urow[:, off + c0 : off + c1],
                        start=False,
                        stop=False,
                    )
                for j, (c0, c1) in enumerate(segs):
                    nc.tensor.matmul(
                        ps[s][:, c0:c1],
                        urow[:, off : off + 128],
                        ones[:, 0 : c1 - c0],
                        start=False,
                        stop=(j == len(segs) - 1),
                    )

            def strip_post(s):
                p = ps[s]
                base = 1 + 9 * s
                scr = post.tile([BLK, STRIPW], F32, tag="scr")
                nc.scalar.activation(
                    scr[:], p[:, 0:STRIPW], EXP, scale=S2E,
                    accum_out=acc[:, base : base + 1],
                )
                pS = post.tile([BLK, STRIPW], F32, tag="pS")
                nc.vector.tensor_scalar(
                    out=pS[:], in0=p[:, 0:STRIPW], scalar1=1.0, scalar2=0.0,
                    op0=MULT, op1=ADD, accum_out=acc[:, base + 3 : base + 4],
                )
                scrd = post.tile([BLK, STRIPW], F32, tag="scrd")
                nc.vector.scalar_tensor_tensor(
                    out=scrd[:], in0=pS[:], scalar=1.0, in1=p[:, 0:STRIPW],
                    op0=MULT, op1=MULT, accum_out=acc[:, base + 6 : base + 7],
                )
                nc.vector.tensor_reduce(
                    acc[:, base + 1 : base + 2], scr[:, 0:128], X, ADD
                )
                nc.vector.tensor_reduce(
                    acc[:, base + 2 : base + 3], scr[:, 1024:1152], X, ADD
                )
                nc.vector.tensor_reduce(
                    acc[:, base + 4 : base + 5], pS[:, 0:128], X, ADD
                )
                nc.vector.tensor_reduce(
                    acc[:, base + 5 : base + 6], pS[:, 1024:1152], X, ADD
                )
                s2d = post.tile([BLK, BLK], F32, tag="s2d")
                nc.vector.scalar_tensor_tensor(
                    out=s2d[:], in0=pS[:, 0:128], scalar=1.0, in1=pS[:, 0:128],
                    op0=MULT, op1=MULT, accum_out=acc[:, base + 7 : base + 8],
                )
                s2h = post.tile([BLK, BLK], F32, tag="s2h")
                nc.vector.scalar_tensor_tensor(
                    out=s2h[:], in0=pS[:, 1024:1152], scalar=1.0,
                    in1=pS[:, 1024:1152],
                    op0=MULT, op1=MULT, accum_out=acc[:, base + 8 : base + 9],
                )

            strip_tail_mms(0)
            strip_post(0)
            strip_tail_mms(1)
            strip_post(1)

            nc.sync.dma_start(out_stats[:], acc[:])
    nc.compile()
    return nc


def _get(name, builder):
    if name not in _cache:
        _cache[name] = builder()
    return _cache[name]


def _run(nc, in_maps, tag):
    if _trace_enabled():
        try:
            import profhook

            profhook.install()
        except Exception:
            pass
        import tempfile

        res = run_bass_kernel_spmd(
            nc, in_maps, list(range(NC_N)), trace=True,
            tmpdir=tempfile.mkdtemp(prefix=f"ktrace_{tag}_"),
        )
        KERNEL_EXEC_NS.append((tag, res.exec_time_ns))
        return res.results
    return run_bass_kernel_spmd(nc, in_maps, list(range(NC_N))).results


def kernel(features):
    x = np.asarray(features).reshape(B, D)
    xbf = x.astype(NP_IN)

    xT_full = np.ascontiguousarray(xbf.T)  # (D, B)
    in_maps = []
    for c in range(NC_N):
        cols = (256 * c + np.arange(UNION)) % B
        xu = xT_full[:, cols].reshape(KCH, BLK, UNION).transpose(1, 0, 2)
        in_maps.append(
            {
                "xT": np.ascontiguousarray(xu),
                "xn": np.ascontiguousarray(
                    xbf[256 * c : 256 * c + 256]
                ).reshape(2, BLK, D),
            }
        )
    nc_k = _get("main", _build_kernel)
    res = _run(nc_k, in_maps, "main")

    # ---- host combine: weighted regions in float64 ----
    FT = E = T1 = T2 = 0.0
    for c in range(NC_N):
        o = res[c]["out_stats"].astype(np.float64)
        FT += o[:, 0].sum()
        for s in range(2):
            b = 1 + 9 * s
            E += o[:, b].sum() - 0.5 * (o[:, b + 1].sum() + o[:, b + 2].sum())
            T1 += o[:, b + 3].sum() - 0.5 * (o[:, b + 4].sum() + o[:, b + 5].sum())
            T2 += o[:, b + 6].sum() - 0.5 * (o[:, b + 7].sum() + o[:, b + 8].sum())
    # closed-form diagonal constants: p_ii = D exactly
    E -= 1024.0 * np.exp(S2E * D)
    T1 -= 1024.0 * D
    T2 -= 1024.0 * D * D

    N = float(N_PAIRS)
    mean_u = -2.0 * T1 / N
    mean = (mean_u + CC) / D
    var_u = (4.0 * T2 - N * mean_u * mean_u) / (N - 1.0)
    std = np.sqrt(var_u) / D
    loss = CC * SS - np.log(E) + np.log(N)
    feat_norm = FT / B

    return (
        np.float32(loss),
        np.float32(feat_norm),
        np.float32(mean),
        np.float32(std),
    )


if __name__ == "__main__":
    f = np.random.default_rng(0).standard_normal((B, 16, 768), dtype=np.float32)
    print(kernel(features=f))
